# revision 22
# baseline (speedup 1.0000x reference)
"""Trainium2 Bass kernel for CustomBertSelfAttention (no head split).

reference:
    q = hs @ Wq + bq; k = hs @ Wk + bk; v = hs @ Wv + bv        # [B,S,D]
    scores = (q @ k^T) / sqrt(64) + mask                         # [B,S,S]
    probs  = softmax(scores, -1)
    out    = probs @ v                                           # [B,S,D]

B=8, S=2048, D=1024.  Sharding: data-parallel over batch, one batch
element per NeuronCore (8 cores), no collectives.

v2 plan — all matmul operands 16-bit (fp16 for hs/W/q/k, bf16 for
exp/v since exp values reach ~2e10 and overflow fp16), which keeps the
PE at 1 cycle/row like fp32r but:
  * everything fits SBUF resident (no DRAM spill round-trips)
  * FWL fast-weight-load applies (fp32r is excluded from FWL)
  * hs transpose runs as REGULAR matmuls (stationary=hs chunk,
    moving=identity) at 1 cycle/row instead of fp32 transpose-mode at
    2 cycles/row -- and real matmuls count as PE-busy for the HAM
    clock gate, so the transpose phase no longer runs cold.
  * rowsum is folded into the context matmul via ones-columns
    appended to v, landing [s-part, 1] in PSUM: no separate rowsum
    pass, no [1,N]->[N,1] transposes before the reciprocal.
  * biases/mask loaded as [c,128] rows (fast contiguous DMA) + one PE
    transpose each, instead of 7-14us DIRECT2D 4-byte scatters.
  * phase 2 emitted software-pipelined (S0 S1 C0 S2 C1 ...) so the PE
    never waits on the exp activation.

Measured numerics (numpy simulation of the exact rounding chain):
max rel-to-max-|out| error 4.2e-3 vs the 2e-2 gate.
"""

import sys

sys.path.insert(0, "/opt/trn_rl_repo")

from contextlib import ExitStack

import numpy as np

import concourse.bass as bass
import concourse.mybir as mybir
import concourse.tile as tile
from concourse import bacc
from concourse.bass_utils import run_bass_kernel_spmd
from concourse.masks import make_identity

B, S, D = 8, 2048, 1024
NCORES = 8
PD = 128            # partition dim
DK = D // PD        # 8 contraction chunks
SC = S // PD        # 16 sequence chunks
NT = 512            # matmul moving-dim tile (one PSUM bank of fp32)
SBLK = 512          # attention s-block
NBLK = S // SBLK    # 4
VW = D + 4          # v row width incl. ones cols for the fused rowsum
F32 = mybir.dt.float32
F16 = mybir.dt.float16
BF16 = mybir.dt.bfloat16
EXP = mybir.ActivationFunctionType.Exp

_compiled_nc = None


def _build():
    nc = bacc.Bacc(
        "TRN2",
        target_bir_lowering=False,
        debug=False,
        num_devices=NCORES,
        enable_asserts=False,
    )
    hs = nc.dram_tensor("hidden_states", [S, D], F32, kind="ExternalInput").ap()
    mask = nc.dram_tensor("attention_mask", [1, S], F32, kind="ExternalInput").ap()
    Wq = nc.dram_tensor("Wq", [D, D], F32, kind="ExternalInput").ap()
    Wk = nc.dram_tensor("Wk", [D, D], F32, kind="ExternalInput").ap()
    Wv = nc.dram_tensor("Wv", [D, D], F32, kind="ExternalInput").ap()
    bq = nc.dram_tensor("bq", [D], F32, kind="ExternalInput").ap()
    bk = nc.dram_tensor("bk", [D], F32, kind="ExternalInput").ap()
    bv = nc.dram_tensor("bv", [D], F32, kind="ExternalInput").ap()
    out = nc.dram_tensor("context", [S, D], F32, kind="ExternalOutput").ap()

    with tile.TileContext(nc) as tc, ExitStack() as ctx:
        persist = ctx.enter_context(tc.tile_pool(name="persist", bufs=1))
        dramp = ctx.enter_context(tc.tile_pool(name="dram", bufs=1, space="DRAM"))

        kT = persist.tile([PD, DK, S], F16)      # [e-part, m, t]
        qT = persist.tile([PD, DK, S], F16)      # [e-part, m, s]
        v_sb = persist.tile([PD, SC, VW], BF16)  # [t-part, c, d | ones]

        mask_sb = persist.tile([PD, SC], F32)    # bias per t-chunk for exp
        bq_sb = persist.tile([PD, DK], F32)
        bk_sb = persist.tile([PD, DK], F32)
        bv_row = persist.tile([PD, D], F32)

        ident = persist.tile([PD, PD], F32)
        make_identity(nc, ident)
        ident16 = persist.tile([PD, PD], F16)

        with ExitStack() as p1:
            hstp = p1.enter_context(tc.tile_pool(name="hsT_pool", bufs=1))
            hsT = hstp.tile([PD, DK, S], F16)    # [d-part, dk, s]

            rowp = p1.enter_context(tc.tile_pool(name="rows", bufs=1))
            bk_row = rowp.tile([DK, PD], F32)
            bq_row = rowp.tile([DK, PD], F32)
            mask_row = rowp.tile([SC, PD], F32)

            w16p = p1.enter_context(tc.tile_pool(name="w16", bufs=2))
            wstp = p1.enter_context(tc.tile_pool(name="wst", bufs=2))
            hsp = p1.enter_context(tc.tile_pool(name="hsload", bufs=3))
            h16p = p1.enter_context(tc.tile_pool(name="hs16", bufs=2))
            ptr = p1.enter_context(tc.tile_pool(name="ptr", bufs=4, space="PSUM"))
            pbias = p1.enter_context(
                tc.tile_pool(name="pbias", bufs=1, space="PSUM")
            )
            junkp = p1.enter_context(
                tc.tile_pool(name="junkp", bufs=1, space="PSUM")
            )
            junksp = p1.enter_context(tc.tile_pool(name="junks", bufs=1))
            pp = p1.enter_context(tc.tile_pool(name="pp", bufs=2, space="PSUM"))

            # DMA emission is software-pipelined: a buffer-rotating pool
            # tile may only get its next DMA writer emitted AFTER the
            # previous epoch's readers are emitted (Tile dep tracking is
            # forward-only), so hs chunk sc+3 is issued when chunk sc is
            # consumed, and W stripe DMAs are issued one per completed
            # W-stripe cast.  ALL input DMAs ride the sync queue in
            # consumption order (hs + Wk stripes interleaved, then Wq,
            # bv, Wv) so nothing competes with the critical prefix.
            hchunks = {}

            def issue_hs_dma(sc):
                if sc >= SC:
                    return
                hchunk = hsp.tile([PD, D], F32, name="hchunk", tag="hchunk")
                nc.sync.dma_start(out=hchunk, in_=hs[sc * PD : (sc + 1) * PD, :])
                hchunks[sc] = hchunk
                if sc == 1:
                    nc.sync.dma_start(
                        out=bk_row, in_=bk.rearrange("(c p) -> c p", c=DK)
                    )
                    nc.sync.dma_start(
                        out=bq_row, in_=bq.rearrange("(c p) -> c p", c=DK)
                    )
                if sc == 8:
                    nc.sync.dma_start(
                        out=mask_row,
                        in_=mask[0, :].rearrange("(c p) -> c p", c=SC),
                    )

            # W stripe queue: column stripes [din, 8dk, 128] so a
            # projection block (m, tb) only needs stripe m
            w_dma_plan = [(Wk, m) for m in range(DK)]
            w_dma_plan += [(Wq, m) for m in range(DK)]
            w_dma_plan += [(Wv, m) for m in range(DK)]
            w_stages = {}
            w_dma_pos = 0

            def issue_w_dma():
                nonlocal w_dma_pos
                if w_dma_pos >= len(w_dma_plan):
                    return
                W, m = w_dma_plan[w_dma_pos]
                w_dma_pos += 1
                wst = wstp.tile([PD, DK, PD], F32, name="wst_t", tag="wst_t")
                nc.sync.dma_start(
                    out=wst,
                    in_=W.rearrange("(dk p) n -> p dk n", p=PD)[
                        :, :, m * PD : (m + 1) * PD
                    ],
                )
                w_stages[(id(W), m)] = wst

            def cast_w_stripe(w16, W, m, noissue=False):
                nc.vector.tensor_copy(
                    out=w16[:, :, m * PD : (m + 1) * PD],
                    in_=w_stages.pop((id(W), m)),
                )
                if not noissue:
                    issue_w_dma()

            issue_hs_dma(0)
            issue_hs_dma(1)
            issue_w_dma()
            issue_hs_dma(2)
            issue_w_dma()
            wk16 = w16p.tile([PD, DK, D], F16, name="w16_t", tag="w16_t")
            wq16 = w16p.tile([PD, DK, D], F16, name="w16_t", tag="w16_t")

            # ---- PE warmup: ~55 junk matmuls on a memset tile (no
            # gpsimd dependency, so the PE is busy from ~1.3us and the HAM
            # clock gate opens before the real work arrives).  The junk
            # drain copy + DCE-keeper DMA are emitted at the END of phase 1
            # so they don't block the hs/W queues behind 55 matmuls.
            jnk16 = junksp.tile([PD, 256], F16, name="jnk16", tag="jnk16")
            nc.vector.memset(jnk16, 0.25)
            nc.vector.tensor_copy(out=ident16, in_=ident)
            warm_ps = junkp.tile([PD, NT], F32, name="warm_ps", tag="warm_ps")
            for _ in range(30):
                nc.tensor.matmul(
                    out=warm_ps[:, 0:256],
                    lhsT=jnk16[:, 0:PD],
                    rhs=jnk16,
                    start=True,
                    stop=True,
                )

            # ---- interleaved transpose + projection emission ----
            def transpose_chunk(sc):
                h16 = h16p.tile([PD, D], F16, name="h16", tag="h16")
                nc.scalar.copy(out=h16, in_=hchunks.pop(sc))
                issue_hs_dma(sc + 3)
                for half in range(2):
                    pst = ptr.tile([PD, 4, PD], F32)
                    for j in range(4):
                        dk = half * 4 + j
                        nc.tensor.matmul(
                            out=pst[:, j, :],
                            lhsT=h16[:, dk * PD : (dk + 1) * PD],
                            rhs=ident16,
                            start=True,
                            stop=True,
                        )
                    nc.vector.tensor_copy(
                        out=hsT[
                            :, half * 4 : (half + 1) * 4, sc * PD : (sc + 1) * PD
                        ],
                        in_=pst,
                    )

            def proj_block(w16, bias_sb, dst, m, tb):
                ps = pp.tile([PD, NT], F32)
                for dk in range(DK):
                    nc.tensor.matmul(
                        out=ps,
                        lhsT=w16[:, dk, m * PD : (m + 1) * PD],
                        rhs=hsT[:, dk, tb * NT : (tb + 1) * NT],
                        start=(dk == 0),
                        stop=(dk == DK - 1),
                    )
                nc.vector.tensor_scalar_add(
                    out=dst[:, m, tb * NT : (tb + 1) * NT],
                    in0=ps,
                    scalar1=bias_sb[:, m : m + 1],
                )

            for sc in range(4):
                transpose_chunk(sc)
                cast_w_stripe(wk16, Wk, sc)
            # bias rows -> [128, DK] via one PE transpose each
            pb = pbias.tile([PD, NT], F32, name="pb_k", tag="pbias")
            nc.tensor.transpose(
                out=pb[:, 0:DK], in_=bk_row, identity=ident[0:DK, 0:DK]
            )
            nc.vector.tensor_copy(out=bk_sb, in_=pb[:, 0:DK])
            pb2 = pbias.tile([PD, NT], F32, name="pb_q", tag="pbias")
            nc.tensor.transpose(
                out=pb2[:, 0:DK], in_=bq_row, identity=ident[0:DK, 0:DK]
            )
            nc.vector.tensor_copy(out=bq_sb, in_=pb2[:, 0:DK])

            # K projection tb0 m-outer: block (m, 0) starts as soon as
            # stripe m lands, tracking the DMA stream instead of waiting
            # for all of Wk.  Remaining transposes interleave.
            proj_block(wk16, bk_sb, kT, 0, 0)
            cast_w_stripe(wk16, Wk, 4)
            proj_block(wk16, bk_sb, kT, 1, 0)
            transpose_chunk(4)
            cast_w_stripe(wk16, Wk, 5)
            proj_block(wk16, bk_sb, kT, 2, 0)
            proj_block(wk16, bk_sb, kT, 3, 0)
            transpose_chunk(5)
            cast_w_stripe(wk16, Wk, 6, noissue=True)  # defer wq issues
            proj_block(wk16, bk_sb, kT, 4, 0)
            proj_block(wk16, bk_sb, kT, 5, 0)
            transpose_chunk(6)
            cast_w_stripe(wk16, Wk, 7, noissue=True)
            proj_block(wk16, bk_sb, kT, 6, 0)
            proj_block(wk16, bk_sb, kT, 7, 0)
            transpose_chunk(7)

            for m in range(DK):
                proj_block(wk16, bk_sb, kT, m, 1)
                if m % 2 == 1:
                    transpose_chunk(8 + m // 2)
            pbm = pbias.tile([PD, NT], F32, name="pb_m", tag="pbias")
            nc.tensor.transpose(
                out=pbm[:, 0:SC], in_=mask_row, identity=ident[0:SC, 0:SC]
            )
            nc.vector.tensor_copy(out=mask_sb, in_=pbm[:, 0:SC])
            for m in range(DK):
                proj_block(wk16, bk_sb, kT, m, 2)
                if m % 2 == 1:
                    transpose_chunk(12 + m // 2)
                    if m == 3:
                        issue_w_dma()  # wq s0 (sync-queue pos after c15)
                    if m == 5:
                        issue_w_dma()  # wq s1
            # bv broadcast after the wq stripes in the queue (slow
            # DIRECT2D replication ~11us; nothing critical behind it)
            bv_bcast = bass.AP(
                tensor=bv.tensor, offset=bv.offset, ap=[[0, PD], *bv.ap]
            )
            nc.sync.dma_start(out=bv_row, in_=bv_bcast)
            for m in range(DK):
                proj_block(wk16, bk_sb, kT, m, 3)

            # Q projection m-outer; each stripe cast auto-issues the next
            # stripe DMA (wq tail, then wv)
            wv16 = w16p.tile([PD, DK, D], F16, name="w16_t", tag="w16_t")
            for m in range(DK):
                cast_w_stripe(wq16, Wq, m)
                for tb in range(4):
                    proj_block(wq16, bq_sb, qT, m, tb)
                if m >= 6:
                    # wv stripe (m-6) was just issued by the wq cast above
                    cast_w_stripe(wv16, Wv, m - 6)
            cast_w_stripe(wv16, Wv, 2)
            cast_w_stripe(wv16, Wv, 3)
            nc.vector.memset(v_sb[:, :, D:VW], 1.0)

            # V projection dt-outer: the first 16 chains only need wv
            # stripes 0-3; stripes 4-7 cast while they run
            for dt in range(D // NT):
                if dt == 1:
                    for m in range(4, DK):
                        cast_w_stripe(wv16, Wv, m)
                for c in range(SC):
                    ps = pp.tile([PD, NT], F32)
                    for dk in range(DK):
                        nc.tensor.matmul(
                            out=ps,
                            lhsT=hsT[:, dk, c * PD : (c + 1) * PD],
                            rhs=wv16[:, dk, dt * NT : (dt + 1) * NT],
                            start=(dk == 0),
                            stop=(dk == DK - 1),
                        )
                    nc.vector.tensor_add(
                        out=v_sb[:, c, dt * NT : (dt + 1) * NT],
                        in0=ps,
                        in1=bv_row[:, dt * NT : (dt + 1) * NT],
                    )

            # junk-warmup drain (kept alive for DCE; emitted late so its
            # deps never gate the startup queues)
            warm_sb = junksp.tile([PD, PD], F32, name="warm_sb", tag="warm_sb")
            nc.vector.tensor_copy(out=warm_sb, in_=warm_ps[:, 0:PD])
            warm_dram = dramp.tile([PD, PD], F32, name="warm_dram", tag="warm_dram")
            nc.sync.dma_start(out=warm_dram[:, :], in_=warm_sb)

        # ---- phase 2: attention, software-pipelined S0 S1 C0 S2 C1 ...
        with (
            tc.tile_pool(name="expp", bufs=3) as epool,
            tc.tile_pool(name="outp", bufs=2) as opool,
            tc.tile_pool(name="rcp", bufs=4) as rpool,
            tc.tile_pool(name="psc", bufs=2, space="PSUM") as psc,
            tc.tile_pool(name="pca", bufs=2, space="PSUM") as pca,
            tc.tile_pool(name="pcb", bufs=2, space="PSUM") as pcb,
            tc.tile_pool(name="pcr", bufs=2, space="PSUM") as pcr,
        ):
            def scores_block(sb):
                exp_sb = epool.tile(
                    [PD, SC, SBLK], BF16, name="exp_sb", tag="exp_sb"
                )
                for tcn in range(SC):
                    ps = psc.tile([PD, SBLK], F32)
                    for dk in range(DK):
                        nc.tensor.matmul(
                            out=ps,
                            lhsT=kT[:, dk, tcn * PD : (tcn + 1) * PD],
                            rhs=qT[:, dk, sb * SBLK : (sb + 1) * SBLK],
                            start=(dk == 0),
                            stop=(dk == DK - 1),
                        )
                    nc.scalar.activation(
                        out=exp_sb[:, tcn, :],
                        in_=ps,
                        func=EXP,
                        scale=0.125,
                        bias=mask_sb[:, tcn : tcn + 1],
                    )
                return exp_sb

            def context_block(sb, exp_sb):
                for ss in range(SBLK // PD):
                    pa = pca.tile([PD, NT], F32)
                    pb = pcb.tile([PD, NT], F32)
                    pr = pcr.tile([PD, NT], F32)
                    for tcn in range(SC):
                        st, sp = (tcn == 0), (tcn == SC - 1)
                        e_sl = exp_sb[:, tcn, ss * PD : (ss + 1) * PD]
                        nc.tensor.matmul(
                            out=pa, lhsT=e_sl, rhs=v_sb[:, tcn, 0:NT],
                            start=st, stop=sp,
                        )
                        nc.tensor.matmul(
                            out=pb, lhsT=e_sl, rhs=v_sb[:, tcn, NT : 2 * NT],
                            start=st, stop=sp,
                        )
                        nc.tensor.matmul(
                            out=pr[:, 0:4], lhsT=e_sl, rhs=v_sb[:, tcn, D:VW],
                            start=st, stop=sp,
                        )
                    recip = rpool.tile([PD, 1], F32, name="recip_t", tag="recip_t")
                    nc.vector.reciprocal(out=recip, in_=pr[:, 0:1])
                    ostage = opool.tile([PD, D], F32)
                    nc.vector.tensor_scalar_mul(
                        out=ostage[:, 0:NT], in0=pa, scalar1=recip
                    )
                    nc.vector.tensor_scalar_mul(
                        out=ostage[:, NT : 2 * NT], in0=pb, scalar1=recip
                    )
                    row = sb * SBLK + ss * PD
                    nc.sync.dma_start(out=out[row : row + PD, :], in_=ostage)

            pending = []
            for sb in range(NBLK):
                e = scores_block(sb)
                pending.append((sb, e))
                if sb >= 1:
                    context_block(*pending.pop(0))
            while pending:
                context_block(*pending.pop(0))

    nc.compile()
    return nc


def _get_compiled():
    global _compiled_nc
    if _compiled_nc is None:
        _compiled_nc = _build()
    return _compiled_nc


def _run(inputs, **kwargs):
    hs = np.asarray(inputs["hidden_states"], dtype=np.float32)
    mask = np.asarray(inputs["attention_mask"], dtype=np.float32)
    ws = {
        k: np.ascontiguousarray(np.asarray(inputs[k], dtype=np.float32))
        for k in ("Wq", "bq", "Wk", "bk", "Wv", "bv")
    }
    nc = _get_compiled()
    in_maps = [
        {
            "hidden_states": np.ascontiguousarray(hs[i]),
            "attention_mask": np.ascontiguousarray(mask[i]),
            **ws,
        }
        for i in range(NCORES)
    ]
    r = run_bass_kernel_spmd(nc, in_maps, list(range(NCORES)), **kwargs)
    out = np.stack([r.results[i]["context"] for i in range(NCORES)], axis=0)
    return out, r


def kernel(**inputs) -> np.ndarray:
    out, _ = _run(inputs)
    return out


if __name__ == "__main__":
    rng = np.random.default_rng(0)
    scale = 1.0 / np.sqrt(D)
    inputs = {
        "hidden_states": rng.standard_normal((B, S, D)).astype(np.float32),
        "attention_mask": np.zeros((B, 1, S), np.float32),
        "Wq": (rng.standard_normal((D, D)) * scale).astype(np.float32),
        "bq": np.zeros(D, np.float32),
        "Wk": (rng.standard_normal((D, D)) * scale).astype(np.float32),
        "bk": np.zeros(D, np.float32),
        "Wv": (rng.standard_normal((D, D)) * scale).astype(np.float32),
        "bv": np.zeros(D, np.float32),
    }
    got = kernel(**inputs)

    hs64 = inputs["hidden_states"].astype(np.float64)
    q = hs64 @ inputs["Wq"].astype(np.float64)
    k = hs64 @ inputs["Wk"].astype(np.float64)
    v = hs64 @ inputs["Wv"].astype(np.float64)
    sc = np.einsum("bsd,btd->bst", q, k) / 8.0
    sc -= sc.max(axis=-1, keepdims=True)
    p = np.exp(sc)
    p /= p.sum(axis=-1, keepdims=True)
    ref = np.einsum("bst,btd->bsd", p, v)
    err = np.abs(got.astype(np.float64) - ref)
    print(
        f"absmax={err.max():.3e} rel_vs_scale={err.max() / np.abs(ref).max():.3e} "
        f"rms_rel={np.sqrt((err**2).mean()) / np.sqrt((ref**2).mean()):.3e}"
    )


# revision 28
# speedup vs baseline: 1.0097x; 1.0097x over previous
"""Trainium2 Bass kernel for CustomBertSelfAttention (no head split).

reference:
    q = hs @ Wq + bq; k = hs @ Wk + bk; v = hs @ Wv + bv        # [B,S,D]
    scores = (q @ k^T) / sqrt(64) + mask                         # [B,S,S]
    probs  = softmax(scores, -1)
    out    = probs @ v                                           # [B,S,D]

B=8, S=2048, D=1024.  Sharding: data-parallel over batch, one batch
element per NeuronCore (8 cores), no collectives.

v2 plan — all matmul operands 16-bit (fp16 for hs/W/q/k, bf16 for
exp/v since exp values reach ~2e10 and overflow fp16), which keeps the
PE at 1 cycle/row like fp32r but:
  * everything fits SBUF resident (no DRAM spill round-trips)
  * FWL fast-weight-load applies (fp32r is excluded from FWL)
  * hs transpose runs as REGULAR matmuls (stationary=hs chunk,
    moving=identity) at 1 cycle/row instead of fp32 transpose-mode at
    2 cycles/row -- and real matmuls count as PE-busy for the HAM
    clock gate, so the transpose phase no longer runs cold.
  * rowsum is folded into the context matmul via ones-columns
    appended to v, landing [s-part, 1] in PSUM: no separate rowsum
    pass, no [1,N]->[N,1] transposes before the reciprocal.
  * biases/mask loaded as [c,128] rows (fast contiguous DMA) + one PE
    transpose each, instead of 7-14us DIRECT2D 4-byte scatters.
  * phase 2 emitted software-pipelined (S0 S1 C0 S2 C1 ...) so the PE
    never waits on the exp activation.

Measured numerics (numpy simulation of the exact rounding chain):
max rel-to-max-|out| error 4.2e-3 vs the 2e-2 gate.
"""

import sys

sys.path.insert(0, "/opt/trn_rl_repo")

from contextlib import ExitStack

import numpy as np

import concourse.bass as bass
import concourse.mybir as mybir
import concourse.tile as tile
from concourse import bacc
from concourse.bass_utils import run_bass_kernel_spmd
from concourse.masks import make_identity

B, S, D = 8, 2048, 1024
NCORES = 8
PD = 128            # partition dim
DK = D // PD        # 8 contraction chunks
SC = S // PD        # 16 sequence chunks
NT = 512            # matmul moving-dim tile (one PSUM bank of fp32)
SBLK = 512          # attention s-block
NBLK = S // SBLK    # 4
VW = D + 4          # v row width incl. ones cols for the fused rowsum
F32 = mybir.dt.float32
F16 = mybir.dt.float16
BF16 = mybir.dt.bfloat16
EXP = mybir.ActivationFunctionType.Exp

_compiled_nc = None


def _build():
    nc = bacc.Bacc(
        "TRN2",
        target_bir_lowering=False,
        debug=False,
        num_devices=NCORES,
        enable_asserts=False,
    )
    hs = nc.dram_tensor("hidden_states", [S, D], F32, kind="ExternalInput").ap()
    mask = nc.dram_tensor("attention_mask", [1, S], F32, kind="ExternalInput").ap()
    Wq = nc.dram_tensor("Wq", [D, D], F32, kind="ExternalInput").ap()
    Wk = nc.dram_tensor("Wk", [D, D], F32, kind="ExternalInput").ap()
    Wv = nc.dram_tensor("Wv", [D, D], F32, kind="ExternalInput").ap()
    bq = nc.dram_tensor("bq", [D], F32, kind="ExternalInput").ap()
    bk = nc.dram_tensor("bk", [D], F32, kind="ExternalInput").ap()
    bv = nc.dram_tensor("bv", [D], F32, kind="ExternalInput").ap()
    out = nc.dram_tensor("context", [S, D], F32, kind="ExternalOutput").ap()

    with tile.TileContext(nc) as tc, ExitStack() as ctx:
        persist = ctx.enter_context(tc.tile_pool(name="persist", bufs=1))
        dramp = ctx.enter_context(tc.tile_pool(name="dram", bufs=1, space="DRAM"))

        kT = persist.tile([PD, DK, S], F16)      # [e-part, m, t]
        qT = persist.tile([PD, DK, S], F16)      # [e-part, m, s]
        v_sb = persist.tile([PD, SC, VW], BF16)  # [t-part, c, d | ones]

        mask_sb = persist.tile([PD, SC], F32)    # bias per t-chunk for exp
        bq_sb = persist.tile([PD, DK], F32)
        bk_sb = persist.tile([PD, DK], F32)
        bv_row = persist.tile([PD, D], F32)

        ident = persist.tile([PD, PD], F32)
        make_identity(nc, ident)
        ident16 = persist.tile([PD, PD], F16)

        with ExitStack() as p1:
            hstp = p1.enter_context(tc.tile_pool(name="hsT_pool", bufs=1))
            hsT = hstp.tile([PD, DK, S], F16)    # [d-part, dk, s]

            rowp = p1.enter_context(tc.tile_pool(name="rows", bufs=1))
            bk_row = rowp.tile([DK, PD], F32)
            bq_row = rowp.tile([DK, PD], F32)
            mask_row = rowp.tile([SC, PD], F32)
            bv_raw = rowp.tile([1, D], F32)
            bv16 = rowp.tile([1, D], F16)
            ones16 = rowp.tile([1, PD], F16)

            w16p = p1.enter_context(tc.tile_pool(name="w16", bufs=2))
            wstp = p1.enter_context(tc.tile_pool(name="wst", bufs=2))
            hsp = p1.enter_context(tc.tile_pool(name="hsload", bufs=3))
            h16p = p1.enter_context(tc.tile_pool(name="hs16", bufs=2))
            ptr = p1.enter_context(tc.tile_pool(name="ptr", bufs=4, space="PSUM"))
            pbias = p1.enter_context(
                tc.tile_pool(name="pbias", bufs=1, space="PSUM")
            )
            junkp = p1.enter_context(
                tc.tile_pool(name="junkp", bufs=1, space="PSUM")
            )
            junksp = p1.enter_context(tc.tile_pool(name="junks", bufs=1))
            pp = p1.enter_context(tc.tile_pool(name="pp", bufs=2, space="PSUM"))

            # DMA emission is software-pipelined: a buffer-rotating pool
            # tile may only get its next DMA writer emitted AFTER the
            # previous epoch's readers are emitted (Tile dep tracking is
            # forward-only), so hs chunk sc+3 is issued when chunk sc is
            # consumed, and W stripe DMAs are issued one per completed
            # W-stripe cast.  ALL input DMAs ride the sync queue in
            # consumption order (hs + Wk stripes interleaved, then Wq,
            # bv, Wv) so nothing competes with the critical prefix.
            hchunks = {}

            def issue_hs_dma(sc):
                if sc >= SC:
                    return
                hchunk = hsp.tile([PD, D], F32, name="hchunk", tag="hchunk")
                nc.sync.dma_start(out=hchunk, in_=hs[sc * PD : (sc + 1) * PD, :])
                hchunks[sc] = hchunk
                if sc == 1:
                    nc.sync.dma_start(
                        out=bk_row, in_=bk.rearrange("(c p) -> c p", c=DK)
                    )
                    nc.sync.dma_start(
                        out=bq_row, in_=bq.rearrange("(c p) -> c p", c=DK)
                    )
                    nc.sync.dma_start(
                        out=bv_raw, in_=bv.rearrange("(c p) -> c p", c=1)
                    )
                if sc == 8:
                    nc.sync.dma_start(
                        out=mask_row,
                        in_=mask[0, :].rearrange("(c p) -> c p", c=SC),
                    )

            # W stripe queue: column stripes [din, 8dk, 128] so a
            # projection block (m, tb) only needs stripe m
            w_dma_plan = [(Wk, m) for m in range(DK)]
            w_dma_plan += [(Wq, m) for m in range(DK)]
            w_dma_plan += [(Wv, m) for m in range(DK)]
            w_stages = {}
            w_dma_pos = 0

            def issue_w_dma():
                nonlocal w_dma_pos
                if w_dma_pos >= len(w_dma_plan):
                    return
                W, m = w_dma_plan[w_dma_pos]
                w_dma_pos += 1
                wst = wstp.tile([PD, DK, PD], F32, name="wst_t", tag="wst_t")
                nc.sync.dma_start(
                    out=wst,
                    in_=W.rearrange("(dk p) n -> p dk n", p=PD)[
                        :, :, m * PD : (m + 1) * PD
                    ],
                )
                w_stages[(id(W), m)] = wst

            def cast_w_stripe(w16, W, m, noissue=False):
                nc.vector.tensor_copy(
                    out=w16[:, :, m * PD : (m + 1) * PD],
                    in_=w_stages.pop((id(W), m)),
                )
                if not noissue:
                    issue_w_dma()

            issue_hs_dma(0)
            issue_hs_dma(1)
            issue_w_dma()
            issue_hs_dma(2)
            issue_w_dma()
            wk16 = w16p.tile([PD, DK, D], F16, name="w16_t", tag="w16_t")
            wq16 = w16p.tile([PD, DK, D], F16, name="w16_t", tag="w16_t")

            # ---- PE warmup: ~55 junk matmuls on a memset tile (no
            # gpsimd dependency, so the PE is busy from ~1.3us and the HAM
            # clock gate opens before the real work arrives).  The junk
            # drain copy + DCE-keeper DMA are emitted at the END of phase 1
            # so they don't block the hs/W queues behind 55 matmuls.
            jnk16 = junksp.tile([PD, 256], F16, name="jnk16", tag="jnk16")
            nc.vector.memset(jnk16, 0.25)
            nc.vector.tensor_copy(out=ident16, in_=ident)
            warm_ps = junkp.tile([PD, NT], F32, name="warm_ps", tag="warm_ps")
            for _ in range(30):
                nc.tensor.matmul(
                    out=warm_ps[:, 0:256],
                    lhsT=jnk16[:, 0:PD],
                    rhs=jnk16,
                    start=True,
                    stop=True,
                )

            # ---- interleaved transpose + projection emission ----
            def transpose_chunk(sc):
                h16 = h16p.tile([PD, D], F16, name="h16", tag="h16")
                nc.scalar.copy(out=h16, in_=hchunks.pop(sc))
                issue_hs_dma(sc + 3)
                for half in range(2):
                    pst = ptr.tile([PD, 4, PD], F32)
                    for j in range(4):
                        dk = half * 4 + j
                        nc.tensor.matmul(
                            out=pst[:, j, :],
                            lhsT=h16[:, dk * PD : (dk + 1) * PD],
                            rhs=ident16,
                            start=True,
                            stop=True,
                        )
                    nc.vector.tensor_copy(
                        out=hsT[
                            :, half * 4 : (half + 1) * 4, sc * PD : (sc + 1) * PD
                        ],
                        in_=pst,
                    )

            def proj_block(w16, bias_sb, dst, m, tb):
                ps = pp.tile([PD, NT], F32)
                for dk in range(DK):
                    nc.tensor.matmul(
                        out=ps,
                        lhsT=w16[:, dk, m * PD : (m + 1) * PD],
                        rhs=hsT[:, dk, tb * NT : (tb + 1) * NT],
                        start=(dk == 0),
                        stop=(dk == DK - 1),
                    )
                nc.vector.tensor_scalar_add(
                    out=dst[:, m, tb * NT : (tb + 1) * NT],
                    in0=ps,
                    scalar1=bias_sb[:, m : m + 1],
                )

            for sc in range(4):
                transpose_chunk(sc)
                cast_w_stripe(wk16, Wk, sc)
            # bias rows -> [128, DK] via one PE transpose each
            pb = pbias.tile([PD, NT], F32, name="pb_k", tag="pbias")
            nc.tensor.transpose(
                out=pb[:, 0:DK], in_=bk_row, identity=ident[0:DK, 0:DK]
            )
            nc.vector.tensor_copy(out=bk_sb, in_=pb[:, 0:DK])
            pb2 = pbias.tile([PD, NT], F32, name="pb_q", tag="pbias")
            nc.tensor.transpose(
                out=pb2[:, 0:DK], in_=bq_row, identity=ident[0:DK, 0:DK]
            )
            nc.vector.tensor_copy(out=bq_sb, in_=pb2[:, 0:DK])
            # bv broadcast across partitions via two rank-1 matmuls
            # (ones-col x bv-row) -- replaces an 11.5us DIRECT2D DMA
            nc.scalar.copy(out=bv16, in_=bv_raw)
            nc.vector.memset(ones16, 1.0)
            for dt in range(D // NT):
                pbv = pbias.tile([PD, NT], F32, name="pb_v", tag="pbias")
                nc.tensor.matmul(
                    out=pbv,
                    lhsT=ones16,
                    rhs=bv16[:, dt * NT : (dt + 1) * NT],
                    start=True,
                    stop=True,
                )
                nc.vector.tensor_copy(
                    out=bv_row[:, dt * NT : (dt + 1) * NT], in_=pbv
                )

            # K projection tb0 m-outer: block (m, 0) starts as soon as
            # stripe m lands, tracking the DMA stream instead of waiting
            # for all of Wk.  Remaining transposes interleave.
            proj_block(wk16, bk_sb, kT, 0, 0)
            cast_w_stripe(wk16, Wk, 4)
            proj_block(wk16, bk_sb, kT, 1, 0)
            transpose_chunk(4)
            cast_w_stripe(wk16, Wk, 5)
            proj_block(wk16, bk_sb, kT, 2, 0)
            proj_block(wk16, bk_sb, kT, 3, 0)
            transpose_chunk(5)
            cast_w_stripe(wk16, Wk, 6, noissue=True)  # defer wq issues
            proj_block(wk16, bk_sb, kT, 4, 0)
            proj_block(wk16, bk_sb, kT, 5, 0)
            transpose_chunk(6)
            cast_w_stripe(wk16, Wk, 7, noissue=True)
            proj_block(wk16, bk_sb, kT, 6, 0)
            proj_block(wk16, bk_sb, kT, 7, 0)
            transpose_chunk(7)

            for m in range(DK):
                proj_block(wk16, bk_sb, kT, m, 1)
                if m % 2 == 1:
                    transpose_chunk(8 + m // 2)
            pbm = pbias.tile([PD, NT], F32, name="pb_m", tag="pbias")
            nc.tensor.transpose(
                out=pbm[:, 0:SC], in_=mask_row, identity=ident[0:SC, 0:SC]
            )
            nc.vector.tensor_copy(out=mask_sb, in_=pbm[:, 0:SC])
            for m in range(DK):
                proj_block(wk16, bk_sb, kT, m, 2)
                if m % 2 == 1:
                    transpose_chunk(12 + m // 2)
                    if m == 3:
                        issue_w_dma()  # wq s0 (sync-queue pos after c15)
                    if m == 5:
                        issue_w_dma()  # wq s1
            for m in range(DK):
                proj_block(wk16, bk_sb, kT, m, 3)

            # Q projection m-outer; each stripe cast auto-issues the next
            # stripe DMA (wq tail, then wv)
            wv16 = w16p.tile([PD, DK, D], F16, name="w16_t", tag="w16_t")
            for m in range(DK):
                cast_w_stripe(wq16, Wq, m)
                for tb in range(4):
                    proj_block(wq16, bq_sb, qT, m, tb)
                if m >= 6:
                    # wv stripe (m-6) was just issued by the wq cast above
                    cast_w_stripe(wv16, Wv, m - 6)
            cast_w_stripe(wv16, Wv, 2)
            cast_w_stripe(wv16, Wv, 3)
            nc.vector.memset(v_sb[:, :, D:VW], 1.0)

            # V projection dt-outer: the first 16 chains only need wv
            # stripes 0-3; stripes 4-7 cast while they run
            for dt in range(D // NT):
                if dt == 1:
                    for m in range(4, DK):
                        cast_w_stripe(wv16, Wv, m)
                for c in range(SC):
                    ps = pp.tile([PD, NT], F32)
                    for dk in range(DK):
                        nc.tensor.matmul(
                            out=ps,
                            lhsT=hsT[:, dk, c * PD : (c + 1) * PD],
                            rhs=wv16[:, dk, dt * NT : (dt + 1) * NT],
                            start=(dk == 0),
                            stop=(dk == DK - 1),
                        )
                    nc.vector.tensor_add(
                        out=v_sb[:, c, dt * NT : (dt + 1) * NT],
                        in0=ps,
                        in1=bv_row[:, dt * NT : (dt + 1) * NT],
                    )

            # junk-warmup drain (kept alive for DCE; emitted late so its
            # deps never gate the startup queues)
            warm_sb = junksp.tile([PD, PD], F32, name="warm_sb", tag="warm_sb")
            nc.vector.tensor_copy(out=warm_sb, in_=warm_ps[:, 0:PD])
            warm_dram = dramp.tile([PD, PD], F32, name="warm_dram", tag="warm_dram")
            nc.sync.dma_start(out=warm_dram[:, :], in_=warm_sb)

        # ---- phase 2: attention, software-pipelined S0 S1 C0 S2 C1 ...
        with (
            tc.tile_pool(name="expp", bufs=3) as epool,
            tc.tile_pool(name="outp", bufs=2) as opool,
            tc.tile_pool(name="rcp", bufs=4) as rpool,
            tc.tile_pool(name="psc", bufs=2, space="PSUM") as psc,
            tc.tile_pool(name="pca", bufs=2, space="PSUM") as pca,
            tc.tile_pool(name="pcb", bufs=2, space="PSUM") as pcb,
            tc.tile_pool(name="pcr", bufs=2, space="PSUM") as pcr,
        ):
            def scores_block(sb):
                exp_sb = epool.tile(
                    [PD, SC, SBLK], BF16, name="exp_sb", tag="exp_sb"
                )
                for tcn in range(SC):
                    ps = psc.tile([PD, SBLK], F32)
                    for dk in range(DK):
                        nc.tensor.matmul(
                            out=ps,
                            lhsT=kT[:, dk, tcn * PD : (tcn + 1) * PD],
                            rhs=qT[:, dk, sb * SBLK : (sb + 1) * SBLK],
                            start=(dk == 0),
                            stop=(dk == DK - 1),
                        )
                    nc.scalar.activation(
                        out=exp_sb[:, tcn, :],
                        in_=ps,
                        func=EXP,
                        scale=0.125,
                        bias=mask_sb[:, tcn : tcn + 1],
                    )
                return exp_sb

            def context_block(sb, exp_sb):
                for ss in range(SBLK // PD):
                    pa = pca.tile([PD, NT], F32)
                    pb = pcb.tile([PD, NT], F32)
                    pr = pcr.tile([PD, NT], F32)
                    for tcn in range(SC):
                        st, sp = (tcn == 0), (tcn == SC - 1)
                        e_sl = exp_sb[:, tcn, ss * PD : (ss + 1) * PD]
                        nc.tensor.matmul(
                            out=pa, lhsT=e_sl, rhs=v_sb[:, tcn, 0:NT],
                            start=st, stop=sp,
                        )
                        nc.tensor.matmul(
                            out=pb, lhsT=e_sl, rhs=v_sb[:, tcn, NT : 2 * NT],
                            start=st, stop=sp,
                        )
                        nc.tensor.matmul(
                            out=pr[:, 0:4], lhsT=e_sl, rhs=v_sb[:, tcn, D:VW],
                            start=st, stop=sp,
                        )
                    recip = rpool.tile([PD, 1], F32, name="recip_t", tag="recip_t")
                    nc.vector.reciprocal(out=recip, in_=pr[:, 0:1])
                    ostage = opool.tile([PD, D], F32)
                    nc.vector.tensor_scalar_mul(
                        out=ostage[:, 0:NT], in0=pa, scalar1=recip
                    )
                    nc.vector.tensor_scalar_mul(
                        out=ostage[:, NT : 2 * NT], in0=pb, scalar1=recip
                    )
                    row = sb * SBLK + ss * PD
                    nc.sync.dma_start(out=out[row : row + PD, :], in_=ostage)

            pending = []
            for sb in range(NBLK):
                e = scores_block(sb)
                pending.append((sb, e))
                if sb >= 1:
                    context_block(*pending.pop(0))
            while pending:
                context_block(*pending.pop(0))

    nc.compile()
    return nc


def _get_compiled():
    global _compiled_nc
    if _compiled_nc is None:
        _compiled_nc = _build()
    return _compiled_nc


def _run(inputs, **kwargs):
    hs = np.asarray(inputs["hidden_states"], dtype=np.float32)
    mask = np.asarray(inputs["attention_mask"], dtype=np.float32)
    ws = {
        k: np.ascontiguousarray(np.asarray(inputs[k], dtype=np.float32))
        for k in ("Wq", "bq", "Wk", "bk", "Wv", "bv")
    }
    nc = _get_compiled()
    in_maps = [
        {
            "hidden_states": np.ascontiguousarray(hs[i]),
            "attention_mask": np.ascontiguousarray(mask[i]),
            **ws,
        }
        for i in range(NCORES)
    ]
    r = run_bass_kernel_spmd(nc, in_maps, list(range(NCORES)), **kwargs)
    out = np.stack([r.results[i]["context"] for i in range(NCORES)], axis=0)
    return out, r


def kernel(**inputs) -> np.ndarray:
    out, _ = _run(inputs)
    return out


if __name__ == "__main__":
    rng = np.random.default_rng(0)
    scale = 1.0 / np.sqrt(D)
    inputs = {
        "hidden_states": rng.standard_normal((B, S, D)).astype(np.float32),
        "attention_mask": np.zeros((B, 1, S), np.float32),
        "Wq": (rng.standard_normal((D, D)) * scale).astype(np.float32),
        "bq": np.zeros(D, np.float32),
        "Wk": (rng.standard_normal((D, D)) * scale).astype(np.float32),
        "bk": np.zeros(D, np.float32),
        "Wv": (rng.standard_normal((D, D)) * scale).astype(np.float32),
        "bv": np.zeros(D, np.float32),
    }
    got = kernel(**inputs)

    hs64 = inputs["hidden_states"].astype(np.float64)
    q = hs64 @ inputs["Wq"].astype(np.float64)
    k = hs64 @ inputs["Wk"].astype(np.float64)
    v = hs64 @ inputs["Wv"].astype(np.float64)
    sc = np.einsum("bsd,btd->bst", q, k) / 8.0
    sc -= sc.max(axis=-1, keepdims=True)
    p = np.exp(sc)
    p /= p.sum(axis=-1, keepdims=True)
    ref = np.einsum("bst,btd->bsd", p, v)
    err = np.abs(got.astype(np.float64) - ref)
    print(
        f"absmax={err.max():.3e} rel_vs_scale={err.max() / np.abs(ref).max():.3e} "
        f"rms_rel={np.sqrt((err**2).mean()) / np.sqrt((ref**2).mean()):.3e}"
    )
